# revision 6
# baseline (speedup 1.0000x reference)
"""Trainium2 Bass kernel for CIAttention (RoPE multi-head attention block).

Full computation:
  q/k/v = x @ W{q,k,v}.T  (per-head split), rope(q), rope(k),
  attn = softmax(q k^T / sqrt(hd)), out = (attn @ v) concat -> @ Wo.T

Sharding over 8 NeuronCores: core c handles batch b=c//2 and head-group
g=c%2 (8 of 16 heads). Megatron-style: o_proj produces partial outputs
that the host sums per batch (the tensor-parallel AllReduce done on host).

All matmuls run in bf16 with fp32 PSUM accumulation. Attention math:
scores are computed transposed (S_T[j,i] = k_j . q_i) so the attn@V
contraction needs no on-chip transposes; softmax skips max-subtraction
(|scores| <= ~7 here so exp is safe) and the row-sum over the partition
axis is produced by an all-ones matmul that also broadcasts it across
partitions for the final normalization multiply.
"""

import numpy as np
import ml_dtypes

import concourse.tile as tile
from concourse import bacc, mybir
from concourse.bass_utils import run_bass_kernel_spmd

BF16 = ml_dtypes.bfloat16

D = 2048          # model dim
S = 2048          # sequence length
B = 4             # batch
H_LOC = 8         # heads per core (16 total / 2 groups)
E_LOC = 1024      # local projection dim (8 heads * 128)
HD = 128          # head dim
INV_SQRT_HD = 1.0 / float(np.sqrt(HD))

_CACHE = {}


def _build_nc():
    f32 = mybir.dt.float32
    bf16 = mybir.dt.bfloat16
    FT = mybir.ActivationFunctionType

    nc = bacc.Bacc("TRN2", debug=False)

    # Inputs, host-swizzled so every DMA has contiguous >=2KB runs.
    xq_d = nc.dram_tensor("xq", [128, 16, S], bf16, kind="ExternalInput")
    xv_d = nc.dram_tensor("xv", [128, 16, 16, 128], bf16, kind="ExternalInput")
    wq_d = nc.dram_tensor("wq", [H_LOC, 128, 16, 128], bf16, kind="ExternalInput")
    wk_d = nc.dram_tensor("wk", [H_LOC, 128, 16, 128], bf16, kind="ExternalInput")
    wv_d = nc.dram_tensor("wv", [128, 16, E_LOC], bf16, kind="ExternalInput")
    wo_d = nc.dram_tensor("wo", [128, 8, D], bf16, kind="ExternalInput")
    cos_d = nc.dram_tensor("cosf", [128, S], bf16, kind="ExternalInput")
    sin_d = nc.dram_tensor("sinf", [128, S], bf16, kind="ExternalInput")
    # Partial output, transposed: outt[e, s]; host adds the two head-group
    # partials per batch and transposes back.
    out_d = nc.dram_tensor("outt", [D, S], f32, kind="ExternalOutput")

    with tile.TileContext(nc) as tc:
        _emit(tc, nc, f32, bf16, FT,
              xq_d, xv_d, wq_d, wk_d, wv_d, wo_d, cos_d, sin_d, out_d)
    nc.compile()
    return nc


def _emit(tc, nc, f32, bf16, FT,
          xq_d, xv_d, wq_d, wk_d, wv_d, wo_d, cos_d, sin_d, out_d):
    from contextlib import ExitStack
    with ExitStack() as top:
        consts = top.enter_context(tc.tile_pool(name="consts", bufs=1))
        # Long-lived activations, pooled separately so each pool's SBUF
        # footprint spans only the phases that need it.
        qk_pool = top.enter_context(tc.tile_pool(name="qk", bufs=1))
        qt_sb = qk_pool.tile([128, H_LOC, S], bf16, tag="qt")
        kt_sb = qk_pool.tile([128, H_LOC, S], bf16, tag="kt")

        ones_sb = consts.tile([128, 128], bf16)
        nc.vector.memset(ones_sb[:], 1.0)

        # ---- Phase 1: Q/K projections (producing q^T/k^T) + RoPE ----
        with tc.tile_pool(name="xq_p", bufs=1) as xpool, \
             tc.tile_pool(name="w1", bufs=3) as wpool, \
             tc.tile_pool(name="ps1", bufs=2, space="PSUM") as ps1, \
             tc.tile_pool(name="ropet", bufs=1) as rt, \
             tc.tile_pool(name="cs", bufs=1) as cs:
            cos_sb = cs.tile([128, S], bf16, tag="cos")
            sin_sb = cs.tile([128, S], bf16, tag="sin")
            nc.sync.dma_start(out=cos_sb[:], in_=cos_d.ap())
            nc.sync.dma_start(out=sin_sb[:], in_=sin_d.ap())
            xsb = xpool.tile([128, 16, S], bf16)
            for dc in range(16):
                nc.sync.dma_start(out=xsb[:, dc, :], in_=xq_d.ap()[:, dc, :])

            for h in range(H_LOC):
                for w_d, out_sb in ((wq_d, qt_sb), (wk_d, kt_sb)):
                    wcol = wpool.tile([128, 16, 128], bf16, tag="wcol")
                    nc.sync.dma_start(out=wcol[:], in_=w_d.ap()[h])
                    for sh in range(2):
                        ssl = slice(sh * 1024, (sh + 1) * 1024)
                        ps = ps1.tile([128, 1024], f32, tag="psqk")
                        for dc in range(16):
                            for nb in range(2):
                                nsl = slice(nb * 512, (nb + 1) * 512)
                                xs = slice(sh * 1024 + nb * 512,
                                           sh * 1024 + (nb + 1) * 512)
                                nc.tensor.matmul(
                                    ps[:, nsl], wcol[:, dc, :], xsb[:, dc, xs],
                                    start=(dc == 0), stop=(dc == 15))
                        # RoPE on [hd, s] layout: rows 0:64 = first half dims.
                        #   out[0:64]  = q1*cos - q2*sin
                        #   out[64:128]= q1*sin + q2*cos
                        # sin_sb is host-prepared as [+sin; -sin] so that after
                        # swapping halves of (ps * sin_sb) the result can be
                        # added partition-aligned (walrus requires matching
                        # start partitions for tensor_tensor ops; the swap is
                        # done with an SBUF->SBUF DMA).
                        tmpA = rt.tile([128, 1024], f32, tag="tA")
                        tmpB = rt.tile([128, 1024], f32, tag="tB")
                        tmpBr = rt.tile([128, 1024], f32, tag="tBr")
                        nc.vector.tensor_mul(tmpA[:], ps[:], cos_sb[:, ssl])
                        nc.vector.tensor_mul(tmpB[:], ps[:], sin_sb[:, ssl])
                        nc.sync.dma_start(out=tmpBr[0:64, :], in_=tmpB[64:128, :])
                        nc.sync.dma_start(out=tmpBr[64:128, :], in_=tmpB[0:64, :])
                        nc.vector.tensor_add(
                            out_sb[:, h, ssl], tmpA[:], tmpBr[:])

        # ---- Phase 2: V projection in natural [s, hd] layout ----
        v_pool = top.enter_context(tc.tile_pool(name="v_pool", bufs=1))
        v_sb = v_pool.tile([128, 16, E_LOC], bf16, tag="v")
        with tc.tile_pool(name="wv_p", bufs=1) as wvp, \
             tc.tile_pool(name="xc", bufs=3) as xcp, \
             tc.tile_pool(name="psv", bufs=2, space="PSUM") as psv:
            wv_sb = wvp.tile([128, 16, E_LOC], bf16)
            for dc in range(16):
                nc.sync.dma_start(out=wv_sb[:, dc, :], in_=wv_d.ap()[:, dc, :])
            for sc in range(16):
                xcol = xcp.tile([128, 16, 128], bf16, tag="xcol")
                nc.sync.dma_start(out=xcol[:], in_=xv_d.ap()[:, sc])
                ps = psv.tile([128, E_LOC], f32, tag="psv")
                for dc in range(16):
                    for nb in range(2):
                        nsl = slice(nb * 512, (nb + 1) * 512)
                        nc.tensor.matmul(
                            ps[:, nsl], xcol[:, dc, :], wv_sb[:, dc, nsl],
                            start=(dc == 0), stop=(dc == 15))
                nc.scalar.copy(v_sb[:, sc, :], ps[:])

        # ---- Phase 3: attention (scores transposed, fused softmax) ----
        aot_pool = top.enter_context(tc.tile_pool(name="aot_pool", bufs=1))
        aot_sb = aot_pool.tile([128, H_LOC, S], bf16, tag="aot")
        with tc.tile_pool(name="at", bufs=2) as atp, \
             tc.tile_pool(name="pss", bufs=2, space="PSUM") as pssp, \
             tc.tile_pool(name="pso", bufs=2, space="PSUM") as psop, \
             tc.tile_pool(name="psr", bufs=2, space="PSUM") as psrp, \
             tc.tile_pool(name="rc_p", bufs=2) as rcp:
            for h in range(H_LOC):
                for ic in range(4):
                    isl = slice(ic * 512, (ic + 1) * 512)
                    attn = atp.tile([128, 16, 512], bf16, tag="attn")
                    so = psop.tile([128, 512], f32, tag="pso")
                    sr = psrp.tile([128, 512], f32, tag="psr")
                    for jc in range(16):
                        jsl = slice(jc * 128, (jc + 1) * 128)
                        ss = pssp.tile([128, 512], f32, tag="pss")
                        nc.tensor.matmul(
                            ss[:], kt_sb[:, h, jsl], qt_sb[:, h, isl],
                            start=True, stop=True)
                        nc.scalar.activation(
                            attn[:, jc, :], ss[:], FT.Exp, scale=INV_SQRT_HD)
                        nc.tensor.matmul(
                            so[:], v_sb[:, jc, h * 128:(h + 1) * 128],
                            attn[:, jc, :], start=(jc == 0), stop=(jc == 15))
                        nc.tensor.matmul(
                            sr[:], ones_sb[:], attn[:, jc, :],
                            start=(jc == 0), stop=(jc == 15))
                    rc = rcp.tile([128, 512], f32, tag="rc")
                    nc.vector.reciprocal(rc[:], sr[:])
                    nc.vector.tensor_mul(aot_sb[:, h, isl], so[:], rc[:])

        # ---- Phase 4: o_proj partial, output transposed [e, s] ----
        with tc.tile_pool(name="wo_p", bufs=1) as wop, \
             tc.tile_pool(name="po", bufs=2, space="PSUM") as pop, \
             tc.tile_pool(name="ost", bufs=3) as ostp:
            wo_sb = wop.tile([128, 8, D], bf16)
            for cc in range(8):
                nc.sync.dma_start(out=wo_sb[:, cc, :], in_=wo_d.ap()[:, cc, :])
            for ec in range(16):
                esl = slice(ec * 128, (ec + 1) * 128)
                for sc4 in range(4):
                    ssl = slice(sc4 * 512, (sc4 + 1) * 512)
                    po = pop.tile([128, 512], f32, tag="po")
                    for cc in range(8):
                        nc.tensor.matmul(
                            po[:], wo_sb[:, cc, esl], aot_sb[:, cc, ssl],
                            start=(cc == 0), stop=(cc == 7))
                    ost = ostp.tile([128, 512], f32, tag="ost")
                    nc.scalar.copy(ost[:], po[:])
                    nc.sync.dma_start(out=out_d.ap()[esl, ssl], in_=ost[:])


def get_nc():
    if "nc" not in _CACHE:
        _CACHE["nc"] = _build_nc()
    return _CACHE["nc"]


def make_in_maps(x, cos, sin, Wq, Wk, Wv, Wo):
    """Host-side shard + swizzle. Returns the 8 per-core input dicts."""
    x = np.asarray(x, np.float32)
    cosT = np.ascontiguousarray(np.asarray(cos, np.float32).T).astype(BF16)
    sinT = np.ascontiguousarray(np.asarray(sin, np.float32).T).astype(BF16)
    cosf = np.ascontiguousarray(np.concatenate([cosT, cosT], 0))  # [128, S]
    # [+sin; -sin]: after the half-swap of ps*sinf, row p<64 holds
    # -q2*sin and row p>=64 holds +q1*sin (see rope comment in _emit).
    sinf = np.ascontiguousarray(np.concatenate([sinT, -sinT], 0))

    per_g = []
    for g in range(2):
        wq_loc = np.asarray(Wq, np.float32)[g * E_LOC:(g + 1) * E_LOC].astype(BF16)
        wk_loc = np.asarray(Wk, np.float32)[g * E_LOC:(g + 1) * E_LOC].astype(BF16)
        wv_loc = np.asarray(Wv, np.float32)[g * E_LOC:(g + 1) * E_LOC].astype(BF16)
        wo_loc = np.asarray(Wo, np.float32)[:, g * E_LOC:(g + 1) * E_LOC].astype(BF16)
        # wq_sw[h, p, c, e] = wq_loc[h*128+e, c*128+p]
        wq_sw = np.ascontiguousarray(
            wq_loc.reshape(H_LOC, 128, 16, 128).transpose(0, 3, 2, 1))
        wk_sw = np.ascontiguousarray(
            wk_loc.reshape(H_LOC, 128, 16, 128).transpose(0, 3, 2, 1))
        # wv_sw[p, c, e] = wv_loc[e, c*128+p]
        wv_sw = np.ascontiguousarray(
            wv_loc.reshape(E_LOC, 16, 128).transpose(2, 1, 0))
        # wo_sw[p, cc, e] = wo_loc[e, cc*128+p]
        wo_sw = np.ascontiguousarray(
            wo_loc.reshape(D, 8, 128).transpose(2, 1, 0))
        per_g.append((wq_sw, wk_sw, wv_sw, wo_sw))

    per_b = []
    for b in range(B):
        xT = np.ascontiguousarray(x[b].astype(BF16).T)  # [d, s]
        xq_sw = np.ascontiguousarray(xT.reshape(16, 128, S).transpose(1, 0, 2))
        xv_sw = np.ascontiguousarray(
            xT.reshape(16, 128, 16, 128).transpose(1, 2, 0, 3))
        per_b.append((xq_sw, xv_sw))

    in_maps = []
    for c in range(8):
        b, g = divmod(c, 2)
        wq_sw, wk_sw, wv_sw, wo_sw = per_g[g]
        xq_sw, xv_sw = per_b[b]
        in_maps.append(dict(xq=xq_sw, xv=xv_sw, wq=wq_sw, wk=wk_sw,
                            wv=wv_sw, wo=wo_sw, cosf=cosf, sinf=sinf))
    return in_maps


def assemble_output(results):
    """results: list of 8 dicts with 'outt' [e, s]. Returns [B, S, D] f32."""
    out = np.empty((B, S, D), np.float32)
    for b in range(B):
        acc = results[2 * b]["outt"] + results[2 * b + 1]["outt"]
        out[b] = acc.T
    return out


def kernel(x, cos, sin, Wq, Wk, Wv, Wo):
    nc = get_nc()
    in_maps = make_in_maps(x, cos, sin, Wq, Wk, Wv, Wo)
    res = run_bass_kernel_spmd(nc, in_maps, core_ids=list(range(8)))
    return assemble_output(res.results)


if __name__ == "__main__":
    # quick self-build check
    get_nc()
    print("built + compiled OK")


# revision 25
# speedup vs baseline: 5813.4237x; 5813.4237x over previous
"""Trainium2 Bass kernel for CIAttention (RoPE multi-head attention block).

Full computation:
  q/k/v = x @ W{q,k,v}.T  (per-head split), rope(q), rope(k),
  attn = softmax(q k^T / sqrt(hd)), out = (attn @ v) concat -> @ Wo.T

Sharding over 8 NeuronCores: core c handles batch b=c//2 and head-group
g=c%2 (8 of 16 heads). Megatron-style: o_proj produces partial outputs
that the host sums per batch (the tensor-parallel AllReduce done on host).

All matmuls run in bf16 with fp32 PSUM accumulation. Attention math:
scores are computed transposed (S_T[j,i] = k_j . q_i) so the attn@V
contraction needs no on-chip transposes; softmax skips max-subtraction
(|scores| <= ~7 here so exp is safe) and the row-sum over the partition
axis is produced by an all-ones matmul that also broadcasts it across
partitions for the final normalization multiply.
"""

import numpy as np
import ml_dtypes

import concourse.tile as tile
from concourse import bacc, mybir
from concourse.bass_utils import run_bass_kernel_spmd

BF16 = ml_dtypes.bfloat16

D = 2048          # model dim
S = 2048          # sequence length
B = 4             # batch
H_LOC = 8         # heads per core (16 total / 2 groups)
E_LOC = 1024      # local projection dim (8 heads * 128)
HD = 128          # head dim
INV_SQRT_HD = 1.0 / float(np.sqrt(HD))

_CACHE = {}

# tuning knobs (overridable for experiments)
KNOBS = dict(
    pss_bufs=3,    # scores psum tiles in flight
    psqk_bufs=3,   # q/k projection psum accumulators
    psv_bufs=2,    # v projection psum accumulators
    po_bufs=2,     # o_proj psum accumulators
    attn_bufs=2,   # per-(h,ic) exp'd score tiles
    wcol_bufs=3,   # streamed q/k weight columns
    emit_rowsum=True,
    rowsum_mode="m128",  # "m128": full-array ones matmuls; "packed": 4x
                         # col-group M=32 ones matmuls + broadcast-sum matmul
    max_phase=4,   # for sim experiments: emit only phases <= this
)


def _build_nc(**overrides):
    knobs = dict(KNOBS)
    knobs.update(overrides)
    f32 = mybir.dt.float32
    bf16 = mybir.dt.bfloat16
    FT = mybir.ActivationFunctionType

    nc = bacc.Bacc("TRN2", debug=False)

    # Inputs, host-swizzled so every DMA has contiguous >=2KB runs.
    xq_d = nc.dram_tensor("xq", [128, 16, S], bf16, kind="ExternalInput")
    wq_d = nc.dram_tensor("wq", [H_LOC, 128, 16, 128], bf16, kind="ExternalInput")
    wk_d = nc.dram_tensor("wk", [H_LOC, 128, 16, 128], bf16, kind="ExternalInput")
    wv_d = nc.dram_tensor("wv", [128, 16, E_LOC], bf16, kind="ExternalInput")
    wo_d = nc.dram_tensor("wo", [128, 8, D], bf16, kind="ExternalInput")
    cos_d = nc.dram_tensor("cosf", [128, S], bf16, kind="ExternalInput")
    sin_d = nc.dram_tensor("sinf", [128, S], bf16, kind="ExternalInput")
    # Partial output, transposed: outt[e, s]; host adds the two head-group
    # partials per batch and transposes back.
    out_d = nc.dram_tensor("outt", [D, S], f32, kind="ExternalOutput")

    with tile.TileContext(nc) as tc:
        _emit(tc, nc, f32, bf16, FT,
              xq_d, wq_d, wk_d, wv_d, wo_d, cos_d, sin_d, out_d, knobs)
    nc.compile()
    return nc


def _emit(tc, nc, f32, bf16, FT,
          xq_d, wq_d, wk_d, wv_d, wo_d, cos_d, sin_d, out_d, knobs):
    from contextlib import ExitStack
    with ExitStack() as top:
        consts = top.enter_context(tc.tile_pool(name="consts", bufs=1))
        # Long-lived activations, pooled separately so each pool's SBUF
        # footprint spans only the phases that need it.
        qk_pool = top.enter_context(tc.tile_pool(name="qk", bufs=1))
        qt_sb = qk_pool.tile([128, H_LOC, S], bf16, tag="qt")
        kt_sb = qk_pool.tile([128, H_LOC, S], bf16, tag="kt")

        ones_sb = consts.tile([128, 128], bf16)
        nc.vector.memset(ones_sb[:], 1.0)
        ones32_sb = consts.tile([128, 32], bf16)
        nc.vector.memset(ones32_sb[:], 1.0)

        # ---- Phase 2: V projection in natural [s, hd] layout ----
        v_pool = top.enter_context(tc.tile_pool(name="v_pool", bufs=1))
        v_sb = v_pool.tile([128, 16, E_LOC], bf16, tag="v")
        with tc.tile_pool(name="wv_p", bufs=1) as wvp, \
             tc.tile_pool(name="xc", bufs=3) as xcp, \
             tc.tile_pool(name="psv", bufs=knobs["psv_bufs"], space="PSUM") as psv:
            wv_sb = wvp.tile([128, 16, E_LOC], bf16)
            for dc in range(16):
                nc.sync.dma_start(out=wv_sb[:, dc, :], in_=wv_d.ap()[:, dc, :])
            for sc in range(16):
                xcol = xcp.tile([128, 16, 128], bf16, tag="xcol")
                nc.sync.dma_start(
                    out=xcol[:], in_=xq_d.ap()[:, :, sc * 128:(sc + 1) * 128])
                ps = psv.tile([128, E_LOC], f32, tag="psv")
                for dc in range(16):
                    for nb in range(2):
                        nsl = slice(nb * 512, (nb + 1) * 512)
                        nc.tensor.matmul(
                            ps[:, nsl], xcol[:, dc, :], wv_sb[:, dc, nsl],
                            start=(dc == 0), stop=(dc == 15))
                nc.scalar.copy(v_sb[:, sc, :], ps[:])

        if knobs["max_phase"] < 2:
            return
        # ---- Phase 1: Q/K projections (producing q^T/k^T) + RoPE ----
        with tc.tile_pool(name="xq_p", bufs=1) as xpool, \
             tc.tile_pool(name="w1", bufs=knobs["wcol_bufs"]) as wpool, \
             tc.tile_pool(name="ps1", bufs=knobs["psqk_bufs"], space="PSUM") as ps1, \
             tc.tile_pool(name="ropet", bufs=1) as rt, \
             tc.tile_pool(name="cs", bufs=1) as cs:
            cos_sb = cs.tile([128, S], bf16, tag="cos")
            sin_sb = cs.tile([128, S], bf16, tag="sin")
            nc.sync.dma_start(out=cos_sb[:], in_=cos_d.ap())
            nc.sync.dma_start(out=sin_sb[:], in_=sin_d.ap())
            xsb = xpool.tile([128, 16, S], bf16)
            for dc in range(16):
                nc.sync.dma_start(out=xsb[:, dc, :], in_=xq_d.ap()[:, dc, :])

            for h in range(H_LOC):
                for w_d, out_sb in ((wq_d, qt_sb), (wk_d, kt_sb)):
                    wcol = wpool.tile([128, 16, 128], bf16, tag="wcol")
                    nc.sync.dma_start(out=wcol[:], in_=w_d.ap()[h])
                    for sh in range(2):
                        ssl = slice(sh * 1024, (sh + 1) * 1024)
                        ps = ps1.tile([128, 1024], f32, tag="psqk")
                        for dc in range(16):
                            for nb in range(2):
                                nsl = slice(nb * 512, (nb + 1) * 512)
                                xs = slice(sh * 1024 + nb * 512,
                                           sh * 1024 + (nb + 1) * 512)
                                nc.tensor.matmul(
                                    ps[:, nsl], wcol[:, dc, :], xsb[:, dc, xs],
                                    start=(dc == 0), stop=(dc == 15))
                        # RoPE on [hd, s] layout: rows 0:64 = first half dims.
                        #   out[0:64]  = q1*cos - q2*sin
                        #   out[64:128]= q1*sin + q2*cos
                        # sin_sb is host-prepared as [+sin; -sin] so that after
                        # swapping halves of (ps * sin_sb) the result can be
                        # added partition-aligned (walrus requires matching
                        # start partitions for tensor_tensor ops, but ACT
                        # copies may move partitions; ACT is idle here).
                        tmpA = rt.tile([128, 1024], f32, tag="tA")
                        tmpB = rt.tile([128, 1024], f32, tag="tB")
                        tmpBr = rt.tile([128, 1024], f32, tag="tBr")
                        nc.vector.tensor_mul(tmpA[:], ps[:], cos_sb[:, ssl])
                        nc.vector.tensor_mul(tmpB[:], ps[:], sin_sb[:, ssl])
                        nc.scalar.copy(tmpBr[0:64, :], tmpB[64:128, :])
                        nc.scalar.copy(tmpBr[64:128, :], tmpB[0:64, :])
                        nc.vector.tensor_add(
                            out_sb[:, h, ssl], tmpA[:], tmpBr[:])

        if knobs["max_phase"] < 3:
            return
        # ---- Phase 3: attention (scores transposed, fused softmax) ----
        aot_pool = top.enter_context(tc.tile_pool(name="aot_pool", bufs=1))
        aot_sb = aot_pool.tile([128, H_LOC, S], bf16, tag="aot")
        with tc.tile_pool(name="at", bufs=knobs["attn_bufs"]) as atp, \
             tc.tile_pool(name="pss", bufs=knobs["pss_bufs"], space="PSUM") as pssp, \
             tc.tile_pool(name="pso", bufs=2, space="PSUM") as psop, \
             tc.tile_pool(name="psr", bufs=2, space="PSUM") as psrp, \
             tc.tile_pool(name="rc_p", bufs=2) as rcp:
            for h in range(H_LOC):
                for ic in range(4):
                    isl = slice(ic * 512, (ic + 1) * 512)
                    attn = atp.tile([128, 16, 512], bf16, tag="attn")
                    so = psop.tile([128, 512], f32, tag="pso")
                    sr = psrp.tile([128, 512], f32, tag="psr")
                    packed = knobs["rowsum_mode"] == "packed"
                    for jc in range(16):
                        jsl = slice(jc * 128, (jc + 1) * 128)
                        ss = pssp.tile([128, 512], f32, tag="pss")
                        nc.tensor.matmul(
                            ss[:], kt_sb[:, h, jsl], qt_sb[:, h, isl],
                            start=True, stop=True)
                        nc.scalar.activation(
                            attn[:, jc, :], ss[:], FT.Exp, scale=INV_SQRT_HD)
                        nc.tensor.matmul(
                            so[:], v_sb[:, jc, h * 128:(h + 1) * 128],
                            attn[:, jc, :], start=(jc == 0), stop=(jc == 15))
                        if knobs["emit_rowsum"] and not packed:
                            nc.tensor.matmul(
                                sr[:], ones_sb[:], attn[:, jc, :],
                                start=(jc == 0), stop=(jc == 15))
                    rc = rcp.tile([128, 512], f32, tag="rc")
                    if packed:
                        # Row-sum via 4 col-group M=32 ones matmuls (the four
                        # groups execute concurrently in the PE array), then
                        # one full-array ones matmul over the copied partials
                        # sums the groups and broadcasts 32*rowsum to all
                        # partitions; the 1/32 folds into the normalize.
                        for jc in range(16):
                            c = jc % 4
                            nc.tensor.matmul(
                                sr[32 * c:32 * (c + 1), :], ones32_sb[:],
                                attn[:, jc, :], start=(jc // 4 == 0),
                                stop=(jc // 4 == 3), tile_position=(0, 32 * c),
                                skip_group_check=True)
                        srs = rcp.tile([128, 512], bf16, tag="srs")
                        nc.scalar.copy(srs[:], sr[:])
                        rb = pssp.tile([128, 512], f32, tag="pss")
                        nc.tensor.matmul(rb[:], ones_sb[:], srs[:],
                                         start=True, stop=True)
                        nc.vector.reciprocal(rc[:], rb[:])
                        nc.vector.scalar_tensor_tensor(
                            aot_sb[:, h, isl], so[:], 32.0, rc[:],
                            mybir.AluOpType.mult, mybir.AluOpType.mult)
                    else:
                        if not knobs["emit_rowsum"]:
                            nc.vector.memset(sr[:], 1.0)
                        nc.vector.reciprocal(rc[:], sr[:])
                        nc.vector.tensor_mul(aot_sb[:, h, isl], so[:], rc[:])

        if knobs["max_phase"] < 4:
            return
        # ---- Phase 4: o_proj partial, output transposed [e, s] ----
        with tc.tile_pool(name="wo_p", bufs=1) as wop, \
             tc.tile_pool(name="po", bufs=knobs["po_bufs"], space="PSUM") as pop, \
             tc.tile_pool(name="ost", bufs=3) as ostp:
            wo_sb = wop.tile([128, 8, D], bf16)
            for cc in range(8):
                nc.sync.dma_start(out=wo_sb[:, cc, :], in_=wo_d.ap()[:, cc, :])
            for ec in range(16):
                esl = slice(ec * 128, (ec + 1) * 128)
                for sc4 in range(4):
                    ssl = slice(sc4 * 512, (sc4 + 1) * 512)
                    po = pop.tile([128, 512], f32, tag="po")
                    for cc in range(8):
                        nc.tensor.matmul(
                            po[:], wo_sb[:, cc, esl], aot_sb[:, cc, ssl],
                            start=(cc == 0), stop=(cc == 7))
                    ost = ostp.tile([128, 512], f32, tag="ost")
                    nc.vector.tensor_copy(ost[:], po[:])
                    nc.sync.dma_start(out=out_d.ap()[esl, ssl], in_=ost[:])


def get_nc():
    if "nc" not in _CACHE:
        _CACHE["nc"] = _build_nc()
    return _CACHE["nc"]


def make_in_maps(x, cos, sin, Wq, Wk, Wv, Wo):
    """Host-side shard + swizzle. Returns the 8 per-core input dicts."""
    x = np.asarray(x, np.float32)
    cosT = np.ascontiguousarray(np.asarray(cos, np.float32).T).astype(BF16)
    sinT = np.ascontiguousarray(np.asarray(sin, np.float32).T).astype(BF16)
    cosf = np.ascontiguousarray(np.concatenate([cosT, cosT], 0))  # [128, S]
    # [+sin; -sin]: after the half-swap of ps*sinf, row p<64 holds
    # -q2*sin and row p>=64 holds +q1*sin (see rope comment in _emit).
    sinf = np.ascontiguousarray(np.concatenate([sinT, -sinT], 0))

    per_g = []
    for g in range(2):
        wq_loc = np.asarray(Wq, np.float32)[g * E_LOC:(g + 1) * E_LOC].astype(BF16)
        wk_loc = np.asarray(Wk, np.float32)[g * E_LOC:(g + 1) * E_LOC].astype(BF16)
        wv_loc = np.asarray(Wv, np.float32)[g * E_LOC:(g + 1) * E_LOC].astype(BF16)
        wo_loc = np.asarray(Wo, np.float32)[:, g * E_LOC:(g + 1) * E_LOC].astype(BF16)
        # wq_sw[h, p, c, e] = wq_loc[h*128+e, c*128+p]
        wq_sw = np.ascontiguousarray(
            wq_loc.reshape(H_LOC, 128, 16, 128).transpose(0, 3, 2, 1))
        wk_sw = np.ascontiguousarray(
            wk_loc.reshape(H_LOC, 128, 16, 128).transpose(0, 3, 2, 1))
        # wv_sw[p, c, e] = wv_loc[e, c*128+p]
        wv_sw = np.ascontiguousarray(
            wv_loc.reshape(E_LOC, 16, 128).transpose(2, 1, 0))
        # wo_sw[p, cc, e] = wo_loc[e, cc*128+p]
        wo_sw = np.ascontiguousarray(
            wo_loc.reshape(D, 8, 128).transpose(2, 1, 0))
        per_g.append((wq_sw, wk_sw, wv_sw, wo_sw))

    per_b = []
    for b in range(B):
        xT = np.ascontiguousarray(x[b].astype(BF16).T)  # [d, s]
        xq_sw = np.ascontiguousarray(xT.reshape(16, 128, S).transpose(1, 0, 2))
        per_b.append(xq_sw)

    in_maps = []
    for c in range(8):
        b, g = divmod(c, 2)
        wq_sw, wk_sw, wv_sw, wo_sw = per_g[g]
        in_maps.append(dict(xq=per_b[b], wq=wq_sw, wk=wk_sw,
                            wv=wv_sw, wo=wo_sw, cosf=cosf, sinf=sinf))
    return in_maps


def assemble_output(results):
    """results: list of 8 dicts with 'outt' [e, s]. Returns [B, S, D] f32."""
    out = np.empty((B, S, D), np.float32)
    for b in range(B):
        acc = results[2 * b]["outt"] + results[2 * b + 1]["outt"]
        out[b] = acc.T
    return out


def _get_runner():
    """Cached sharded-jit runner (replicates bass2jax.run_bass_via_pjrt's
    shard_map path, with output zero-buffers created on device)."""
    if "runner" in _CACHE:
        return _CACHE["runner"]
    import jax
    import jax.numpy as jnp
    from jax.sharding import Mesh, PartitionSpec, NamedSharding
    from jax.experimental.shard_map import shard_map
    from concourse import bass2jax
    from concourse.bass2jax import _bass_exec_p, partition_id_tensor

    nc = get_nc()
    bass2jax.install_neuronx_cc_hook()
    n_cores = 8
    partition_name = nc.partition_id_tensor.name if nc.partition_id_tensor else None
    in_names, out_names, out_avals, zero_shapes = [], [], [], []
    for alloc in nc.m.functions[0].allocations:
        if not isinstance(alloc, mybir.MemoryLocationSet):
            continue
        name = alloc.memorylocations[0].name
        if alloc.kind == "ExternalInput":
            if name != partition_name:
                in_names.append(name)
        elif alloc.kind == "ExternalOutput":
            shape = tuple(alloc.tensor_shape)
            dtype = mybir.dt.np(alloc.dtype)
            out_names.append(name)
            out_avals.append(jax.core.ShapedArray(shape, dtype))
            zero_shapes.append((shape, dtype))

    n_params = len(in_names)
    n_outs = len(out_avals)
    all_in_names = list(in_names) + list(out_names)
    if partition_name is not None:
        all_in_names.append(partition_name)

    def _body(*args):
        operands = list(args)
        if partition_name is not None:
            operands.append(partition_id_tensor())
        outs = _bass_exec_p.bind(
            *operands,
            out_avals=tuple(out_avals),
            in_names=tuple(all_in_names),
            out_names=tuple(out_names),
            lowering_input_output_aliases=(),
            sim_require_finite=True,
            sim_require_nnan=True,
            nc=nc,
        )
        return tuple(outs)

    devices = jax.devices()[:n_cores]
    mesh = Mesh(np.asarray(devices), ("core",))
    in_specs = (PartitionSpec("core"),) * (n_params + n_outs)
    out_specs = (PartitionSpec("core"),) * n_outs
    donate = tuple(range(n_params, n_params + n_outs))
    sharded = jax.jit(
        shard_map(_body, mesh=mesh, in_specs=in_specs, out_specs=out_specs,
                  check_rep=False),
        donate_argnums=donate,
        keep_unused=True,
    )
    sharding = NamedSharding(mesh, PartitionSpec("core"))
    zero_fn = jax.jit(
        lambda: tuple(
            jnp.zeros((n_cores * shp[0], *shp[1:]), dt)
            for shp, dt in zero_shapes),
        out_shardings=tuple(sharding for _ in zero_shapes),
    )

    # Per-batch pair reduction on device: partial(core 2b) + partial(core
    # 2b+1), transposed back to [s, e] and cast bf16 (one rounding of the
    # final output; halves the slow host<->terminal fetch).
    pair_add = jax.jit(lambda a, b: (a + b).T.astype(jnp.bfloat16))

    def run(in_maps):
        # The axon tunnel is slow (~90 MB/s) but device-to-device copies are
        # fast, so upload each unique host array once and replicate on device.
        uploaded = {}  # id(np array) -> {core: device_array}

        def shard_for(arr, c):
            ent = uploaded.setdefault(id(arr), {})
            if c in ent:
                return ent[c]
            if ent:
                src = next(iter(ent.values()))
                a = jax.device_put(src, devices[c])
            else:
                a = jax.device_put(arr, devices[c])
            ent[c] = a
            return a

        args = []
        for name in in_names:
            shards = [shard_for(np.asarray(m[name]), c)
                      for c, m in enumerate(in_maps)]
            a0 = np.asarray(in_maps[0][name])
            gshape = (n_cores * a0.shape[0], *a0.shape[1:])
            args.append(jax.make_array_from_single_device_arrays(
                gshape, sharding, shards))
        args.extend(zero_fn())
        outs = sharded(*args)
        out0 = outs[0]
        summed = []
        for b in range(n_cores // 2):
            s0 = out0.addressable_shards[2 * b].data
            s1 = out0.addressable_shards[2 * b + 1].data
            s1m = jax.device_put(s1, devices[2 * b])
            summed.append(pair_add(s0, s1m))
        for s in summed:
            try:
                s.copy_to_host_async()
            except Exception:
                pass
        return [np.asarray(s) for s in summed]

    _CACHE["runner"] = run
    return run


def kernel(x, cos, sin, Wq, Wk, Wv, Wo):
    in_maps = make_in_maps(x, cos, sin, Wq, Wk, Wv, Wo)
    run = _get_runner()
    partials = run(in_maps)  # 4 arrays [s, e] bf16 (per batch)
    out = np.empty((B, S, D), np.float32)
    for b in range(B):
        out[b] = partials[b]
    return out


if __name__ == "__main__":
    # quick self-build check
    get_nc()
    print("built + compiled OK")
